# revision 13
# baseline (speedup 1.0000x reference)
"""Trainium2 Bass kernel for nn_AttentionFFM — v6.

Per token (b, k), v = x[b, :, k]:
    e_ij  = exp(w_ij v_i v_j),  out_i = v_i * (sum_j e_ij v_j) / (sum_j e_ij)

Layout per k-PAIR: partition p = (k2, j), free = (i outer 64, b inner 128).

Findings baked in from v2-v5 traces:
  - DVE tensor_tensor bf16 2x (4.42us per [128,8192] op) is the floor; the
    3 big multiplies (s, z, u) dominate.  PE diag-matmul z and big GPSIMD
    offloads both LOSE (LDWEIGHTS is real time on HW; heavy GPSIMD/PE SBUF
    streaming degrades concurrent DVE ops ~1.2-1.7x).  Light GPSIMD usage
    (~<4us/pair) is safe.
  - Engine queues are in-order: issue order IS the schedule.  v5's 4-slot
    rotation + issue reordering achieves ~98% DVE duty; remaining losses
    were startup (DMA ordering) and tail.

v6 structure:
  DVE:  s; z for i in [0, ZA); u; recip; o1.
  GPS:  z for i in [ZA, ZA+ZG) (contiguous slice); final v_i multiply.
  ACT:  one big exp over [0, ZA+ZG); per-i exp(s*wcol_i) via per-partition
        scale AP for the NFOLD trailing i's (z never materialized there);
        one-time broadcast-copy building w_rep on-chip (saves startup DMA).
  PE:   D/T j-reductions.
  DMA:  xi(0) first on sync+scalar queues; per-pair partial output DMA.
"""

import sys
from contextlib import ExitStack

import numpy as np

if "/opt/trn_rl_repo" not in sys.path:
    sys.path.insert(0, "/opt/trn_rl_repo")

import concourse.bass as bass
import concourse.tile as tile
from concourse import bacc, mybir
from concourse.bass import MemorySpace
from concourse.bass_utils import run_bass_kernel_spmd

_NEFF_CACHE_DIR = "/tmp/bass_neff_cache"


def _install_neff_cache():
    import hashlib
    import shutil

    from concourse import bass_utils as _bu

    if getattr(_bu.compile_bir_kernel, "_is_cached_wrapper", False):
        return

    _orig = _bu.compile_bir_kernel

    _volatile = {"ant_traceback", "filename", "lineno", "kernel_name"}

    def _strip(obj):
        if isinstance(obj, dict):
            return {k: _strip(v) for k, v in obj.items() if k not in _volatile}
        if isinstance(obj, list):
            return [_strip(v) for v in obj]
        return obj

    def _key(bir_json):
        import orjson

        try:
            normalized = orjson.dumps(_strip(orjson.loads(bir_json)))
        except Exception:
            normalized = bir_json
        return hashlib.sha256(normalized).hexdigest()[:32]

    def _cached(bir_json, tmpdir, neff_name="file.neff"):
        import os as _os

        try:
            _os.makedirs(_NEFF_CACHE_DIR, exist_ok=True)
            p = _os.path.join(_NEFF_CACHE_DIR, _key(bir_json) + ".neff")
            dst = _os.path.join(tmpdir, neff_name)
            if _os.path.exists(p):
                shutil.copy(p, dst)
                return dst
            out = _orig(bir_json, tmpdir, neff_name)
            try:
                shutil.copy(out, p)
            except Exception:
                pass
            return out
        except Exception:
            return _orig(bir_json, tmpdir, neff_name)

    _cached._is_cached_wrapper = True
    _bu.compile_bir_kernel = _cached
    try:
        import concourse.bass2jax as _b2j

        if hasattr(_b2j, "compile_bir_kernel"):
            _b2j.compile_bir_kernel = _cached
    except Exception:
        pass


_install_neff_cache()

B, M, K = 1024, 64, 16
NCORES = 8
BL = B // NCORES
NP = K // 2  # k-pairs

_CACHE = {}
LAST_RESULTS = None
TRACE = False
P_LIMIT = NP
LINEARIZE = False

import os as _os

ZG = int(_os.environ.get("ZG", "12"))  # i's with z on GPSIMD
NFOLD = int(_os.environ.get("NFOLD", "9"))  # i's with z folded into ACT scale
OUTM_GPS = _os.environ.get("OUTM_GPS", "1") == "1"
NSLOT = 4

# legacy knobs referenced by test.py
S_ENGINE = "vector"
X_COPY = False
NBUF = 2


def _build():
    nc = bacc.Bacc(
        "TRN2",
        target_bir_lowering=False,
        debug=False,
        num_devices=NCORES,
    )
    bf16 = mybir.dt.bfloat16
    f32 = mybir.dt.float32
    mult = mybir.AluOpType.mult

    FREE = M * BL  # 8192
    ZA = M - ZG - NFOLD  # i's with z on DVE
    ZB = ZA + ZG  # exp-big covers [0, ZB)
    assert 0 <= ZA and ZB <= M

    xtj_in = nc.declare_dram_parameter("xtj", [128, NP * BL], bf16, isOutput=False)
    xflat_in = nc.declare_dram_parameter("xflat", [K, M * BL], bf16, isOutput=False)
    xq_in = nc.declare_dram_parameter("xq", [BL, NP * 2 * M], bf16, isOutput=False)
    sel_in = nc.declare_dram_parameter("sel", [128, 2], bf16, isOutput=False)
    wcol_in = nc.declare_dram_parameter("wcol", [128, M], f32, isOutput=False)
    out_ext = nc.declare_dram_parameter("out", [BL, NP * 2 * M], f32, isOutput=True)

    with tile.TileContext(nc, linearize=LINEARIZE) as tc, ExitStack() as ctx:
        const = ctx.enter_context(tc.tile_pool(name="const", bufs=1))
        big = ctx.enter_context(tc.tile_pool(name="big", bufs=1))
        ps = ctx.enter_context(tc.tile_pool(name="ps", bufs=1, space=MemorySpace.PSUM))

        # --- tiles ---
        xtj = const.tile([128, NP, BL], bf16)
        sel = const.tile([128, 2], bf16)
        wcol = const.tile([128, M], f32)
        w_rep = const.tile([128, ZB, BL], bf16)
        xq = const.tile([BL, NP, 2 * M], bf16)
        obuf = const.tile([BL, NP, 2 * M], f32)

        xe_t = [
            big.tile([128, FREE], bf16, tag=f"xe{r}", name=f"xe{r}")
            for r in range(NSLOT)
        ]
        su_t = [
            big.tile([128, FREE], bf16, tag=f"su{r}", name=f"su{r}")
            for r in range(NSLOT)
        ]
        z_t = [
            big.tile([128, ZB * BL], bf16, tag=f"z{c}", name=f"z{c}") for c in range(2)
        ]
        rd_t = [big.tile([BL, 2 * M], f32, tag=f"rd{c}", name=f"rd{c}") for c in range(2)]
        o1_t = [big.tile([BL, 2 * M], f32, tag=f"o1{c}", name=f"o1{c}") for c in range(2)]
        D_ps = [ps.tile([BL, 2 * M], f32, tag=f"D{c}", name=f"D{c}") for c in range(2)]
        T_ps = [ps.tile([BL, 2 * M], f32, tag=f"T{c}", name=f"T{c}") for c in range(2)]

        def bcast_xi(q, c0=0, c1=FREE):
            r = q % NSLOT
            for k2 in range(2):
                row = xflat_in[2 * q + k2 : 2 * q + k2 + 1, c0:c1]
                src = bass.AP(tensor=row.tensor, offset=row.offset,
                              ap=[[0, 64], [1, c1 - c0]])
                eng = nc.sync if k2 == 0 else nc.scalar
                eng.dma_start(out=xe_t[r][64 * k2 : 64 * (k2 + 1), c0:c1], in_=src)

        # --- startup DMAs ---
        # Queue discipline: sync and scalar DMA queues round-robin the ring
        # bandwidth, so keep tiny transfers first and never put a compute op
        # (which blocks descriptor generation) between DMA issues.
        nc.sync.dma_start(out=wcol[:, :], in_=wcol_in[:, :])
        nc.sync.dma_start(out=sel[:, :], in_=sel_in[:, :])
        nc.scalar.dma_start(
            out=xtj[:, :, :], in_=xtj_in[:, :].rearrange("p (q b) -> p q b", b=BL)
        )
        bcast_xi(0, 0, FREE // 2)
        bcast_xi(0, FREE // 2, FREE)
        for q in range(1, min(NSLOT, P_LIMIT)):
            bcast_xi(q)
        nc.scalar.dma_start(
            out=xq[:, :, :], in_=xq_in[:, :].rearrange("p (q c) -> p q c", c=2 * M)
        )
        # build w_rep on-chip: broadcast wcol over b (one-time ACT copy);
        # issued after all startup DMA descriptor generation.
        nc.scalar.activation(
            out=w_rep[:, :, :],
            in_=wcol[:, :ZB].unsqueeze(2).broadcast_to((128, ZB, BL)),
            func=mybir.ActivationFunctionType.Copy,
        )

        def xtj_view(q, i0=0, i1=M):  # [128, i1-i0, BL]: v_j per (partition, b)
            return xtj[:, q, :].unsqueeze(1).broadcast_to((128, i1 - i0, BL))

        def s3_of(q):
            return su_t[q % NSLOT][:, :].rearrange("p (i b) -> p i b", b=BL)

        def e3_of(q):
            return xe_t[q % NSLOT][:, :].rearrange("p (i b) -> p i b", b=BL)

        def issue_s(q):
            nc.vector.tensor_tensor(
                out=s3_of(q),
                in0=xtj_view(q),
                in1=e3_of(q),  # xi lives in the xe tile at this point
                op=mult,
            )

        def issue_z(q):
            s = su_t[q % NSLOT]
            s3 = s3_of(q)
            z = z_t[q % 2]
            wr = w_rep[:, :, :].rearrange("p i b -> p (i b)")
            if ZA > 0:
                nc.vector.tensor_tensor(
                    out=z[:, : ZA * BL],
                    in0=s[:, : ZA * BL],
                    in1=wr[:, : ZA * BL],
                    op=mult,
                )
            if ZG > 0:
                nc.gpsimd.tensor_tensor(
                    out=z[:, ZA * BL :],
                    in0=s[:, ZA * BL : ZB * BL],
                    in1=wr[:, ZA * BL :],
                    op=mult,
                )
            e = xe_t[q % NSLOT]
            # folded exps first: they only need s (not z) — keeps the last
            # pair's tail chain short.
            for i in range(ZB, M):
                nc.scalar.activation(
                    out=e3_of(q)[:, i, :],
                    in_=s3[:, i, :],
                    func=mybir.ActivationFunctionType.Exp,
                    scale=wcol[:, i : i + 1],
                )
            nc.scalar.activation(
                out=e[:, : ZB * BL],
                in_=z[:, :],
                func=mybir.ActivationFunctionType.Exp,
            )

        def issue_u(q):
            nc.vector.tensor_tensor(
                out=s3_of(q),
                in0=e3_of(q),
                in1=xtj_view(q),
                op=mult,
            )

        def issue_dt(q):
            c = q % 2
            e3 = e3_of(q)
            s3 = s3_of(q)
            for i in range(M):
                nc.tensor.matmul(
                    D_ps[c][:, 2 * i : 2 * i + 2], e3[:, i, :], sel[:, :],
                    start=True, stop=True,
                )
                nc.tensor.matmul(
                    T_ps[c][:, 2 * i : 2 * i + 2], s3[:, i, :], sel[:, :],
                    start=True, stop=True,
                )

        def issue_tail(q):
            c = q % 2
            rd = rd_t[c]
            nc.vector.reciprocal_approx_fast(out=rd[:, :], in_=D_ps[c][:, :])
            o1 = o1_t[c]
            nc.vector.tensor_tensor(
                out=o1[:, :], in0=T_ps[c][:, :], in1=rd[:, :], op=mult
            )
            eng = nc.gpsimd if OUTM_GPS else nc.vector
            eng.tensor_tensor(
                out=obuf[:, q, :], in0=o1[:, :], in1=xq[:, q, :], op=mult
            )
            nc.sync.dma_start(
                out=out_ext[:, 2 * M * q : 2 * M * (q + 1)], in_=obuf[:, q, :]
            )

        def issue_z_split(q):
            # last pair: halve z/exp so u/DT can start while the second
            # exp half is still running (shortens the serial tail chain)
            h = ZA // 2
            s = su_t[q % NSLOT]
            s3 = s3_of(q)
            z = z_t[q % 2]
            e = xe_t[q % NSLOT]
            wr = w_rep[:, :, :].rearrange("p i b -> p (i b)")
            for i in range(ZB, M):
                nc.scalar.activation(
                    out=e3_of(q)[:, i, :],
                    in_=s3[:, i, :],
                    func=mybir.ActivationFunctionType.Exp,
                    scale=wcol[:, i : i + 1],
                )
            nc.vector.tensor_tensor(
                out=z[:, : h * BL], in0=s[:, : h * BL], in1=wr[:, : h * BL],
                op=mult,
            )
            nc.scalar.activation(
                out=e[:, : h * BL], in_=z[:, : h * BL],
                func=mybir.ActivationFunctionType.Exp,
            )
            nc.vector.tensor_tensor(
                out=z[:, h * BL : ZA * BL], in0=s[:, h * BL : ZA * BL],
                in1=wr[:, h * BL : ZA * BL], op=mult,
            )
            if ZG > 0:
                nc.gpsimd.tensor_tensor(
                    out=z[:, ZA * BL :], in0=s[:, ZA * BL : ZB * BL],
                    in1=wr[:, ZA * BL :], op=mult,
                )
            nc.scalar.activation(
                out=e[:, h * BL : ZB * BL], in_=z[:, h * BL :],
                func=mybir.ActivationFunctionType.Exp,
            )
            return h

        def issue_u_dt_split(q, h):
            c = q % 2
            e3 = e3_of(q)
            s3 = s3_of(q)
            nc.vector.tensor_tensor(
                out=s3[:, :h, :], in0=e3[:, :h, :], in1=xtj_view(q, 0, h),
                op=mult,
            )
            nc.vector.tensor_tensor(
                out=s3[:, h:, :], in0=e3[:, h:, :], in1=xtj_view(q, h, M),
                op=mult,
            )
            for i in range(M):
                nc.tensor.matmul(
                    D_ps[c][:, 2 * i : 2 * i + 2], e3[:, i, :], sel[:, :],
                    start=True, stop=True,
                )
                nc.tensor.matmul(
                    T_ps[c][:, 2 * i : 2 * i + 2], s3[:, i, :], sel[:, :],
                    start=True, stop=True,
                )

        _last_h = [M // 2]
        P = P_LIMIT
        # pair 0: split s into i-halves so compute starts on the first
        # half-broadcast of xi(0)
        nc.vector.tensor_tensor(
            out=s3_of(0)[:, : M // 2, :],
            in0=xtj_view(0, 0, M // 2),
            in1=e3_of(0)[:, : M // 2, :],
            op=mult,
        )
        nc.vector.tensor_tensor(
            out=s3_of(0)[:, M // 2 :, :],
            in0=xtj_view(0, M // 2, M),
            in1=e3_of(0)[:, M // 2 :, :],
            op=mult,
        )
        issue_z(0)
        if P > 1:
            issue_s(1)
        for q in range(P):
            last_h = None
            if q + 1 < P:
                if q + 1 == P - 1:
                    last_h = issue_z_split(q + 1)
                else:
                    issue_z(q + 1)
            if q + 2 < P:
                issue_s(q + 2)
            if q == P - 1 and P > 1:
                issue_u_dt_split(q, _last_h[0])
            else:
                issue_u(q)
                issue_dt(q)
            if last_h is not None:
                _last_h[0] = last_h
            issue_tail(q)
            if q + NSLOT < P:
                bcast_xi(q + NSLOT)

    nc.compile()
    return nc


def _get_nc():
    if "nc" not in _CACHE:
        _CACHE["nc"] = _build()
    return _CACHE["nc"]


def _prep_core(xc, shared):
    """CPU-side layout prep for one core. xc [BL, M, K] f32."""
    import ml_dtypes

    bf = ml_dtypes.bfloat16
    xb = xc.astype(bf)
    xt = xb.transpose(2, 1, 0)  # [k, j, b]
    xtj = (
        xt.reshape(NP, 2, M, BL).transpose(1, 2, 0, 3).reshape(128, NP * BL)
    )  # [(k2 j), (q b)]
    xflat = xt.reshape(K, M * BL)  # [k, (i b)] (i outer, b inner)
    # xq[b, q, (i, k2)] = x[b, i, 2q + k2]
    xq = xb.reshape(BL, M, NP, 2).transpose(0, 2, 1, 3).reshape(BL, NP * 2 * M)
    m = {
        "xtj": np.ascontiguousarray(xtj),
        "xflat": np.ascontiguousarray(xflat),
        "xq": np.ascontiguousarray(xq),
    }
    m.update(shared)
    return m


def kernel(x, vk):
    global LAST_RESULTS
    x = np.ascontiguousarray(np.asarray(x), dtype=np.float32)
    vk = np.ascontiguousarray(np.asarray(vk), dtype=np.float32)
    assert x.shape == (B, M, K) and vk.shape[0] == M

    import ml_dtypes

    bf = ml_dtypes.bfloat16
    wb = (vk @ vk.T).astype(bf)
    sel = np.zeros((128, 2), dtype=bf)
    sel[:64, 0] = 1
    sel[64:, 1] = 1
    # wcol[(k2 j), i] = w[j, i]
    wcol = np.ascontiguousarray(np.concatenate([wb, wb], axis=0).astype(np.float32))
    shared = {"wcol": wcol, "sel": sel}

    in_maps = [_prep_core(x[i * BL : (i + 1) * BL], shared) for i in range(NCORES)]

    nc = _get_nc()
    res = run_bass_kernel_spmd(nc, in_maps, core_ids=list(range(NCORES)), trace=TRACE)
    LAST_RESULTS = res
    # out[b, i, 2q+k2] = obuf[b, q, i, k2]
    out = np.concatenate(
        [
            np.asarray(res.results[i]["out"])
            .reshape(BL, NP, M, 2)
            .transpose(0, 2, 1, 3)
            .reshape(BL, M, K)
            for i in range(NCORES)
        ],
        axis=0,
    )
    return np.ascontiguousarray(out, dtype=np.float32)
